# revision 11
# baseline (speedup 1.0000x reference)
"""Fused cross-attention kernel for Trainium2, data-parallel over batch on 8 cores.

Per core (one batch element):
  tn   = LayerNorm(text)                      (gamma folded into Wk/Wv on host)
  Q^T  = Wq^T @ X^T   (X^T pre-transposed on host, bf16)
  K^T  = Wk^T @ tn^T,  V = tn @ Wv            (tn^T via PE transpose)
  per q-tile of 128 rows, per head:
    S    = Q_h^T.T @ K_h^T                    (q on partitions, l on free)
    E    = exp(S * scale)                     (no max-sub: |S*scale| < 2)
    A    = (E * mask) / sum(E * mask)         (masked softmax)
    C^T  = V_h.T'd via lhsT=V_h, rhs=A^T      (A^T via PE transpose)
  out  = concat_h(C) @ Wo                     (natural layout, DMA out)
"""

import sys

sys.path.insert(0, "/opt/trn_rl_repo")

import numpy as np
import ml_dtypes

import concourse.bass as bass
import concourse.mybir as mybir
import concourse.tile as tile
from concourse import bacc
from concourse.bass_utils import run_bass_kernel_spmd
from concourse.masks import make_identity

N_CORES = 8
B, T, S_, D, L, H = 8, 64, 196, 512, 77, 4
DH = D // H  # 128
NQ = T * S_  # 12544
LN_EPS = 1e-6
SCALE = float(DH) ** -0.5
P = 128
NCH = D // P  # 4 chunks of the feature dim

F32 = mybir.dt.float32
BF16 = mybir.dt.bfloat16

LAST_RESULTS = None  # BassKernelResults of the most recent run (for test harness)
_PROGRAM_CACHE = {}


def build_program(nq=NQ):
    """One SPMD program; all 8 cores run it on their own batch element."""
    nc = bacc.Bacc("TRN2", target_bir_lowering=False, debug=False, num_devices=N_CORES)

    xt = nc.dram_tensor("xt", [D, nq], BF16, kind="ExternalInput").ap()
    text = nc.dram_tensor("text", [P, D], F32, kind="ExternalInput").ap()
    maskv = nc.dram_tensor("maskv", [P, 1], F32, kind="ExternalInput").ap()
    maskb = nc.dram_tensor("maskb", [P], BF16, kind="ExternalInput").ap()
    negcnt = nc.dram_tensor("negcnt", [P, 1], F32, kind="ExternalInput").ap()
    wq = nc.dram_tensor("wq", [D, D], BF16, kind="ExternalInput").ap()
    wk = nc.dram_tensor("wk", [D, D], BF16, kind="ExternalInput").ap()
    wv = nc.dram_tensor("wv", [D, D], BF16, kind="ExternalInput").ap()
    wo = nc.dram_tensor("wo", [D, D], BF16, kind="ExternalInput").ap()
    out = nc.dram_tensor("out", [nq, D], F32, kind="ExternalOutput").ap()

    ntiles = nq // P
    # q-tile groups of up to 4 (512 q rows per Q-projection pass)
    groups = []
    t0 = 0
    while t0 < ntiles:
        gt = min(4, ntiles - t0)
        groups.append((t0, gt))
        t0 += gt

    with tile.TileContext(nc) as tc:
        with (
            tc.tile_pool(name="const", bufs=1) as const,
            tc.tile_pool(name="xtp", bufs=2) as xtp,
            tc.tile_pool(name="qtp", bufs=2) as qtp,
            tc.tile_pool(name="attp", bufs=3) as attp,
            tc.tile_pool(name="smalls", bufs=24) as smalls,
            tc.tile_pool(name="outp", bufs=3) as outp,
            tc.tile_pool(name="ps_qt", bufs=2, space="PSUM") as ps_qt,
            tc.tile_pool(name="ps_sc", bufs=2, space="PSUM") as ps_sc,
            tc.tile_pool(name="ps_at", bufs=2, space="PSUM") as ps_at,
            tc.tile_pool(name="ps_ctx", bufs=1, space="PSUM") as ps_ctx,
            tc.tile_pool(name="ps_out", bufs=1, space="PSUM") as ps_out,
        ):
            # ---- constants / prolog ----
            ident = const.tile([P, P], BF16)
            make_identity(nc, ident)

            wq_sb = const.tile([P, NCH, D], BF16, tag="wq")
            wk_sb = const.tile([P, NCH, D], BF16, tag="wk")
            wv_sb = const.tile([P, NCH, D], BF16, tag="wv")
            wo_sb = const.tile([P, NCH, D], BF16, tag="wo")
            for w_sb, w_dram in ((wq_sb, wq), (wk_sb, wk), (wv_sb, wv), (wo_sb, wo)):
                nc.sync.dma_start(
                    out=w_sb[:], in_=w_dram.rearrange("(c p) n -> p c n", p=P)
                )

            text_sb = const.tile([P, D], F32, tag="text")
            nc.sync.dma_start(out=text_sb[:], in_=text)
            maskv_sb = const.tile([P, 1], F32, tag="maskv")
            nc.sync.dma_start(out=maskv_sb[:], in_=maskv)
            negcnt_sb = const.tile([P, 1], F32, tag="negcnt")
            nc.sync.dma_start(out=negcnt_sb[:], in_=negcnt)
            maskb_sb = const.tile([P, P], BF16, tag="maskb")
            maskb_bcast = bass.AP(
                tensor=maskb.tensor, offset=maskb.offset, ap=[[0, P]] + maskb.ap
            )
            nc.gpsimd.dma_start(out=maskb_sb[:], in_=maskb_bcast)

            # ---- LayerNorm of text (77 valid rows; pad rows are zeros) ----
            stats = smalls.tile([P, 6], F32, tag="bnstats")
            nc.vector.bn_stats(out=stats[:], in_=text_sb[:])
            mv = smalls.tile([P, 2], F32, tag="bnaggr")
            nc.vector.bn_aggr(out=mv[:], in_=stats[:])
            eps_sb = smalls.tile([P, 1], F32, tag="eps")
            nc.vector.memset(eps_sb[:], LN_EPS)
            std = smalls.tile([P, 1], F32, tag="std")
            nc.scalar.activation(
                std[:], mv[:, 1:2], mybir.ActivationFunctionType.Sqrt, bias=eps_sb[:]
            )
            rstd = smalls.tile([P, 1], F32, tag="rstd")
            nc.vector.reciprocal(rstd[:], std[:])
            tn_sb = const.tile([P, D], BF16, tag="tn")
            nc.vector.tensor_scalar(
                out=tn_sb[:],
                in0=text_sb[:],
                scalar1=mv[:, 0:1],
                scalar2=rstd[:],
                op0=mybir.AluOpType.subtract,
                op1=mybir.AluOpType.mult,
            )

            # ---- tn^T (D on partitions), K^T, V ----
            tnT_sb = const.tile([P, NCH, P], BF16, tag="tnT")
            for c in range(NCH):
                ps_t = ps_at.tile([P, P], BF16, tag="ps_trb")
                nc.tensor.transpose(ps_t[:], tn_sb[:, c * P : (c + 1) * P], ident[:])
                nc.scalar.copy(tnT_sb[:, c, :], ps_t[:])

            kt_sb = const.tile([P, H, P], BF16, tag="kt")
            for dch in range(NCH):
                ps_k = ps_sc.tile([P, P], F32, tag="ps_s")
                for kc in range(NCH):
                    nc.tensor.matmul(
                        ps_k[:],
                        wk_sb[:, kc, dch * P : (dch + 1) * P],
                        tnT_sb[:, kc, :],
                        start=(kc == 0),
                        stop=(kc == NCH - 1),
                    )
                nc.vector.tensor_mul(kt_sb[:, dch, :], ps_k[:], maskb_sb[:])

            ps_v = ps_qt.tile([P, D], F32, tag="ps_q")
            for kc in range(NCH):
                nc.tensor.matmul(
                    ps_v[:],
                    tnT_sb[:, kc, :],
                    wv_sb[:, kc, :],
                    start=(kc == 0),
                    stop=(kc == NCH - 1),
                )
            v_sb = const.tile([P, D], BF16, tag="v")
            nc.vector.tensor_scalar_mul(v_sb[:], ps_v[:], maskv_sb[:])

            # ---- main loop over q-tile groups ----
            for t0, gt in groups:
                qg = gt * P
                q0 = t0 * P

                xt_sb = xtp.tile([P, NCH, qg], BF16, tag="xt")
                nc.sync.dma_start(
                    out=xt_sb[:],
                    in_=xt.rearrange("(c p) q -> p c q", p=P)[:, :, q0 : q0 + qg],
                )

                qt_sb = qtp.tile([P, H, qg], BF16, tag="qt")
                for dch in range(NCH):
                    ps_q = ps_qt.tile([P, qg], F32, tag="ps_q")
                    for kc in range(NCH):
                        nc.tensor.matmul(
                            ps_q[:],
                            wq_sb[:, kc, dch * P : (dch + 1) * P],
                            xt_sb[:, kc, :],
                            start=(kc == 0),
                            stop=(kc == NCH - 1),
                        )
                    nc.scalar.copy(qt_sb[:, dch, :], ps_q[:])

                for t in range(gt):
                    tq = slice(t * P, (t + 1) * P)
                    ps_s = ps_sc.tile([P, D], F32, tag="ps_s")
                    for h in range(H):
                        nc.tensor.matmul(
                            ps_s[:, h * P : (h + 1) * P],
                            qt_sb[:, h, tq], kt_sb[:, h, :],
                            start=True, stop=True,
                        )
                    exp_sb = attp.tile([P, H, P], BF16, tag="exp")
                    nc.scalar.activation(
                        exp_sb[:], ps_s[:].rearrange("p (c n) -> p c n", c=H),
                        mybir.ActivationFunctionType.Exp, scale=SCALE,
                    )
                    sumexp = smalls.tile([P, H], F32, tag="sumexp")
                    nc.vector.reduce_sum(
                        out=sumexp[:], in_=exp_sb[:], axis=mybir.AxisListType.X
                    )
                    sumadj = smalls.tile([P, H], F32, tag="sumadj")
                    nc.vector.tensor_scalar_add(sumadj[:], sumexp[:], negcnt_sb[:])
                    recip = smalls.tile([P, H], F32, tag="recip")
                    nc.vector.reciprocal(recip[:], sumadj[:])
                    attn_sb = attp.tile([P, H, P], BF16, tag="attn")
                    ps_a = ps_at.tile([P, H * P], BF16, tag="ps_trb")
                    for h in range(H):
                        nc.gpsimd.tensor_scalar_mul(
                            attn_sb[:, h, :], exp_sb[:, h, :], recip[:, h : h + 1]
                        )
                        nc.tensor.transpose(
                            ps_a[:, h * P : (h + 1) * P], attn_sb[:, h, :], ident[:]
                        )
                    attnT_sb = attp.tile([P, H, P], BF16, tag="attnT")
                    nc.vector.tensor_copy(
                        attnT_sb[:], ps_a[:].rearrange("p (c n) -> p c n", c=H)
                    )
                    ps_c = ps_ctx.tile([P, D], F32, tag="ps_c")
                    ctxT_sb = attp.tile([P, H, P], BF16, tag="ctxT")
                    for h in range(H):
                        nc.tensor.matmul(
                            ps_c[:, h * P : (h + 1) * P],
                            v_sb[:, h * P : (h + 1) * P],
                            attnT_sb[:, h, :],
                            start=True, stop=True,
                        )
                    nc.vector.tensor_copy(ctxT_sb[:], ps_c[:].rearrange("p (c n) -> p c n", c=H))

                    ps_o = ps_out.tile([P, D], F32, tag="ps_o")
                    for h in range(H):
                        nc.tensor.matmul(
                            ps_o[:],
                            ctxT_sb[:, h, :],
                            wo_sb[:, h, :],
                            start=(h == 0),
                            stop=(h == H - 1),
                        )
                    out_sb = outp.tile([P, D], F32, tag="out")
                    nc.scalar.copy(out_sb[:], ps_o[:])
                    nc.sync.dma_start(
                        out=out[q0 + t * P : q0 + (t + 1) * P, :], in_=out_sb[:]
                    )

    nc.compile()
    return nc


def _get_program(nq=NQ):
    if nq not in _PROGRAM_CACHE:
        _PROGRAM_CACHE[nq] = build_program(nq)
    return _PROGRAM_CACHE[nq]


def prep_core_inputs(visual_feat, text_feat, token_mask, wq, wk, wv, wo,
                     ln_gamma, ln_beta):
    """Host-side prep: shard over batch, fold gamma, transpose X, cast bf16."""
    vf = np.ascontiguousarray(visual_feat.reshape(B, -1, D))
    wk2 = (ln_gamma[:, None] * wk).astype(np.float32)
    wv2 = (ln_gamma[:, None] * wv).astype(np.float32)
    wq_b = wq.astype(ml_dtypes.bfloat16)
    wk_b = wk2.astype(ml_dtypes.bfloat16)
    wv_b = wv2.astype(ml_dtypes.bfloat16)
    wo_b = wo.astype(ml_dtypes.bfloat16)

    in_maps = []
    for b in range(B):
        xt = np.ascontiguousarray(vf[b].T).astype(ml_dtypes.bfloat16)
        text = np.zeros((P, D), np.float32)
        text[:L] = text_feat[b]
        m = token_mask[b].astype(np.float32)
        maskv = np.zeros((P, 1), np.float32)
        maskv[:L, 0] = m
        maskb = np.zeros((P,), ml_dtypes.bfloat16)
        maskb[:L] = m.astype(ml_dtypes.bfloat16)
        negcnt = np.full((P, 1), -(P - float(m.sum())), np.float32)
        in_maps.append({
            "xt": xt, "text": text, "maskv": maskv, "maskb": maskb,
            "negcnt": negcnt,
            "wq": wq_b, "wk": wk_b, "wv": wv_b, "wo": wo_b,
        })
    # LN beta correction: beta affects scores only via a softmax-invariant
    # per-row constant, and the output via a constant row added everywhere.
    out_corr = (ln_beta.astype(np.float64) @ wv2.astype(np.float64)
                @ wo.astype(np.float64)).astype(np.float32)
    return in_maps, out_corr


def kernel(visual_feat, text_feat, token_mask, Wq, Wk, Wv, Wo, ln_gamma, ln_beta):
    global LAST_RESULTS
    visual_feat = np.asarray(visual_feat, np.float32)
    text_feat = np.asarray(text_feat, np.float32)
    token_mask = np.asarray(token_mask)

    in_maps, out_corr = prep_core_inputs(
        visual_feat, text_feat, token_mask,
        np.asarray(Wq, np.float32), np.asarray(Wk, np.float32),
        np.asarray(Wv, np.float32), np.asarray(Wo, np.float32),
        np.asarray(ln_gamma, np.float32), np.asarray(ln_beta, np.float32),
    )
    nc = _get_program()
    res = run_bass_kernel_spmd(nc, in_maps, core_ids=list(range(N_CORES)))
    LAST_RESULTS = res
    out = np.stack([res.results[b]["out"] for b in range(B)], axis=0)
    if np.any(out_corr):
        out = out + out_corr[None, None, :]
    return out.reshape(B, T, S_, D)


# revision 12
# speedup vs baseline: 3.5239x; 3.5239x over previous
"""Fused cross-attention kernel for Trainium2, data-parallel over batch on 8 cores.

Per core (one batch element):
  tn   = LayerNorm(text)                      (gamma folded into Wk/Wv on host)
  Q^T  = Wq^T @ X^T   (X^T pre-transposed on host, bf16)
  K^T  = Wk^T @ tn^T,  V = tn @ Wv            (tn^T via PE transpose)
  per q-tile of 128 rows, per head:
    S    = Q_h^T.T @ K_h^T                    (q on partitions, l on free)
    E    = exp(S * scale)                     (no max-sub: |S*scale| < 2)
    A    = (E * mask) / sum(E * mask)         (masked softmax)
    C^T  = V_h.T'd via lhsT=V_h, rhs=A^T      (A^T via PE transpose)
  out  = concat_h(C) @ Wo                     (natural layout, DMA out)
"""

import sys

sys.path.insert(0, "/opt/trn_rl_repo")

import numpy as np
import ml_dtypes

import concourse.bass as bass
import concourse.mybir as mybir
import concourse.tile as tile
from concourse import bacc
from concourse.bass_utils import run_bass_kernel_spmd
from concourse.masks import make_identity

N_CORES = 8
B, T, S_, D, L, H = 8, 64, 196, 512, 77, 4
DH = D // H  # 128
NQ = T * S_  # 12544
LN_EPS = 1e-6
SCALE = float(DH) ** -0.5
P = 128
NCH = D // P  # 4 chunks of the feature dim

F32 = mybir.dt.float32
BF16 = mybir.dt.bfloat16

LAST_RESULTS = None  # BassKernelResults of the most recent run (for test harness)
_PROGRAM_CACHE = {}


def build_program(nq=NQ):
    """One SPMD program; all 8 cores run it on their own batch element."""
    nc = bacc.Bacc("TRN2", target_bir_lowering=False, debug=False, num_devices=N_CORES)

    xt = nc.dram_tensor("xt", [D, nq], BF16, kind="ExternalInput").ap()
    text = nc.dram_tensor("text", [P, D], F32, kind="ExternalInput").ap()
    maskv = nc.dram_tensor("maskv", [P, 1], F32, kind="ExternalInput").ap()
    maskb = nc.dram_tensor("maskb", [P], BF16, kind="ExternalInput").ap()
    negcnt = nc.dram_tensor("negcnt", [P, 1], F32, kind="ExternalInput").ap()
    wq = nc.dram_tensor("wq", [D, D], BF16, kind="ExternalInput").ap()
    wk = nc.dram_tensor("wk", [D, D], BF16, kind="ExternalInput").ap()
    wv = nc.dram_tensor("wv", [D, D], BF16, kind="ExternalInput").ap()
    wo = nc.dram_tensor("wo", [D, D], BF16, kind="ExternalInput").ap()
    out = nc.dram_tensor("out", [nq, D], F32, kind="ExternalOutput").ap()

    ntiles = nq // P
    # q-tile groups of up to 4 (512 q rows per Q-projection pass)
    groups = []
    t0 = 0
    while t0 < ntiles:
        gt = min(4, ntiles - t0)
        groups.append((t0, gt))
        t0 += gt

    with tile.TileContext(nc) as tc:
        with (
            tc.tile_pool(name="const", bufs=1) as const,
            tc.tile_pool(name="xtp", bufs=2) as xtp,
            tc.tile_pool(name="qtp", bufs=2) as qtp,
            tc.tile_pool(name="attp", bufs=3) as attp,
            tc.tile_pool(name="smalls", bufs=24) as smalls,
            tc.tile_pool(name="outp", bufs=3) as outp,
            tc.tile_pool(name="ps_qt", bufs=2, space="PSUM") as ps_qt,
            tc.tile_pool(name="ps_sc", bufs=2, space="PSUM") as ps_sc,
            tc.tile_pool(name="ps_at", bufs=2, space="PSUM") as ps_at,
            tc.tile_pool(name="ps_ctx", bufs=1, space="PSUM") as ps_ctx,
            tc.tile_pool(name="ps_out", bufs=1, space="PSUM") as ps_out,
        ):
            # ---- constants / prolog ----
            ident = const.tile([P, P], BF16)
            make_identity(nc, ident)

            wq_sb = const.tile([P, NCH, D], BF16, tag="wq")
            wk_sb = const.tile([P, NCH, D], BF16, tag="wk")
            wv_sb = const.tile([P, NCH, D], BF16, tag="wv")
            wo_sb = const.tile([P, NCH, D], BF16, tag="wo")
            for w_sb, w_dram in ((wq_sb, wq), (wk_sb, wk), (wv_sb, wv), (wo_sb, wo)):
                nc.sync.dma_start(
                    out=w_sb[:], in_=w_dram.rearrange("(c p) n -> p c n", p=P)
                )

            text_sb = const.tile([P, D], F32, tag="text")
            nc.sync.dma_start(out=text_sb[:], in_=text)
            maskv_sb = const.tile([P, 1], F32, tag="maskv")
            nc.sync.dma_start(out=maskv_sb[:], in_=maskv)
            negcnt_sb = const.tile([P, 1], F32, tag="negcnt")
            nc.sync.dma_start(out=negcnt_sb[:], in_=negcnt)
            maskb_sb = const.tile([P, P], BF16, tag="maskb")
            maskb_bcast = bass.AP(
                tensor=maskb.tensor, offset=maskb.offset, ap=[[0, P]] + maskb.ap
            )
            nc.gpsimd.dma_start(out=maskb_sb[:], in_=maskb_bcast)

            # ---- LayerNorm of text (77 valid rows; pad rows are zeros) ----
            stats = smalls.tile([P, 6], F32, tag="bnstats")
            nc.vector.bn_stats(out=stats[:], in_=text_sb[:])
            mv = smalls.tile([P, 2], F32, tag="bnaggr")
            nc.vector.bn_aggr(out=mv[:], in_=stats[:])
            eps_sb = smalls.tile([P, 1], F32, tag="eps")
            nc.vector.memset(eps_sb[:], LN_EPS)
            std = smalls.tile([P, 1], F32, tag="std")
            nc.scalar.activation(
                std[:], mv[:, 1:2], mybir.ActivationFunctionType.Sqrt, bias=eps_sb[:]
            )
            rstd = smalls.tile([P, 1], F32, tag="rstd")
            nc.vector.reciprocal(rstd[:], std[:])
            tn_sb = const.tile([P, D], BF16, tag="tn")
            nc.vector.tensor_scalar(
                out=tn_sb[:],
                in0=text_sb[:],
                scalar1=mv[:, 0:1],
                scalar2=rstd[:],
                op0=mybir.AluOpType.subtract,
                op1=mybir.AluOpType.mult,
            )

            # ---- tn^T (D on partitions), K^T, V ----
            tnT_sb = const.tile([P, NCH, P], BF16, tag="tnT")
            for c in range(NCH):
                ps_t = ps_at.tile([P, P], BF16, tag="ps_trb")
                nc.tensor.transpose(ps_t[:], tn_sb[:, c * P : (c + 1) * P], ident[:])
                nc.scalar.copy(tnT_sb[:, c, :], ps_t[:])

            kt_sb = const.tile([P, H, L], BF16, tag="kt")
            for dch in range(NCH):
                ps_k = ps_sc.tile([P, L], F32, tag="ps_s")
                for kc in range(NCH):
                    nc.tensor.matmul(
                        ps_k[:],
                        wk_sb[:, kc, dch * P : (dch + 1) * P],
                        tnT_sb[:, kc, :L],
                        start=(kc == 0),
                        stop=(kc == NCH - 1),
                    )
                nc.vector.tensor_mul(kt_sb[:, dch, :], ps_k[:], maskb_sb[:, :L])

            ps_v = ps_qt.tile([P, D], F32, tag="ps_q")
            for kc in range(NCH):
                nc.tensor.matmul(
                    ps_v[:],
                    tnT_sb[:, kc, :],
                    wv_sb[:, kc, :],
                    start=(kc == 0),
                    stop=(kc == NCH - 1),
                )
            v_sb = const.tile([P, D], BF16, tag="v")
            nc.vector.tensor_scalar_mul(v_sb[:], ps_v[:], maskv_sb[:])

            # ---- main loop over q-tile groups ----
            for t0, gt in groups:
                qg = gt * P
                q0 = t0 * P

                xt_sb = xtp.tile([P, NCH, qg], BF16, tag="xt")
                nc.sync.dma_start(
                    out=xt_sb[:],
                    in_=xt.rearrange("(c p) q -> p c q", p=P)[:, :, q0 : q0 + qg],
                )

                qt_sb = qtp.tile([P, H, qg], BF16, tag="qt")
                for dch in range(NCH):
                    ps_q = ps_qt.tile([P, qg], F32, tag="ps_q")
                    for kc in range(NCH):
                        nc.tensor.matmul(
                            ps_q[:],
                            wq_sb[:, kc, dch * P : (dch + 1) * P],
                            xt_sb[:, kc, :],
                            start=(kc == 0),
                            stop=(kc == NCH - 1),
                        )
                    nc.scalar.copy(qt_sb[:, dch, :], ps_q[:])

                for t in range(gt):
                    tq = slice(t * P, (t + 1) * P)
                    ps_s = ps_sc.tile([P, H * L], F32, tag="ps_s")
                    for h in range(H):
                        nc.tensor.matmul(
                            ps_s[:, h * L : (h + 1) * L],
                            qt_sb[:, h, tq], kt_sb[:, h, :],
                            start=True, stop=True,
                        )
                    exp_sb = attp.tile([P, H, L], BF16, tag="exp")
                    nc.scalar.activation(
                        exp_sb[:], ps_s[:].rearrange("p (c n) -> p c n", c=H),
                        mybir.ActivationFunctionType.Exp, scale=SCALE,
                    )
                    sumexp = smalls.tile([P, H], F32, tag="sumexp")
                    nc.vector.reduce_sum(
                        out=sumexp[:], in_=exp_sb[:], axis=mybir.AxisListType.X
                    )
                    sumadj = smalls.tile([P, H], F32, tag="sumadj")
                    nc.vector.tensor_scalar_add(sumadj[:], sumexp[:], negcnt_sb[:])
                    recip = smalls.tile([P, H], F32, tag="recip")
                    nc.vector.reciprocal_approx_fast(recip[:], sumadj[:])
                    attn_sb = attp.tile([P, H, L], BF16, tag="attn")
                    nc.vector.tensor_mul(
                        attn_sb[:], exp_sb[:], recip[:].to_broadcast([P, H, L])
                    )
                    ps_a = ps_at.tile([P, H * P], BF16, tag="ps_trb")
                    for h in range(H):
                        nc.tensor.transpose(
                            ps_a[:L, h * P : (h + 1) * P], attn_sb[:, h, :], ident[:]
                        )
                    attnT_sb = attp.tile([P, H, P], BF16, tag="attnT")
                    nc.vector.tensor_copy(
                        attnT_sb[:L], ps_a[:L].rearrange("p (c n) -> p c n", c=H)
                    )
                    ps_c = ps_ctx.tile([P, D], F32, tag="ps_c")
                    ctxT_sb = attp.tile([P, H, P], BF16, tag="ctxT")
                    for h in range(H):
                        nc.tensor.matmul(
                            ps_c[:, h * P : (h + 1) * P],
                            v_sb[:L, h * P : (h + 1) * P],
                            attnT_sb[:L, h, :],
                            start=True, stop=True,
                        )
                    nc.scalar.copy(ctxT_sb[:], ps_c[:].rearrange("p (c n) -> p c n", c=H))

                    ps_o = ps_out.tile([P, D], F32, tag="ps_o")
                    for h in range(H):
                        nc.tensor.matmul(
                            ps_o[:],
                            ctxT_sb[:, h, :],
                            wo_sb[:, h, :],
                            start=(h == 0),
                            stop=(h == H - 1),
                        )
                    out_sb = outp.tile([P, D], F32, tag="out")
                    nc.scalar.copy(out_sb[:], ps_o[:])
                    nc.sync.dma_start(
                        out=out[q0 + t * P : q0 + (t + 1) * P, :], in_=out_sb[:]
                    )

    nc.compile()
    return nc


def _get_program(nq=NQ):
    if nq not in _PROGRAM_CACHE:
        _PROGRAM_CACHE[nq] = build_program(nq)
    return _PROGRAM_CACHE[nq]


def prep_core_inputs(visual_feat, text_feat, token_mask, wq, wk, wv, wo,
                     ln_gamma, ln_beta):
    """Host-side prep: shard over batch, fold gamma, transpose X, cast bf16."""
    vf = np.ascontiguousarray(visual_feat.reshape(B, -1, D))
    wk2 = (ln_gamma[:, None] * wk).astype(np.float32)
    wv2 = (ln_gamma[:, None] * wv).astype(np.float32)
    wq_b = wq.astype(ml_dtypes.bfloat16)
    wk_b = wk2.astype(ml_dtypes.bfloat16)
    wv_b = wv2.astype(ml_dtypes.bfloat16)
    wo_b = wo.astype(ml_dtypes.bfloat16)

    in_maps = []
    for b in range(B):
        xt = np.ascontiguousarray(vf[b].T).astype(ml_dtypes.bfloat16)
        text = np.zeros((P, D), np.float32)
        text[:L] = text_feat[b]
        m = token_mask[b].astype(np.float32)
        maskv = np.zeros((P, 1), np.float32)
        maskv[:L, 0] = m
        maskb = np.zeros((P,), ml_dtypes.bfloat16)
        maskb[:L] = m.astype(ml_dtypes.bfloat16)
        negcnt = np.full((P, 1), -(L - float(m.sum())), np.float32)
        in_maps.append({
            "xt": xt, "text": text, "maskv": maskv, "maskb": maskb,
            "negcnt": negcnt,
            "wq": wq_b, "wk": wk_b, "wv": wv_b, "wo": wo_b,
        })
    # LN beta correction: beta affects scores only via a softmax-invariant
    # per-row constant, and the output via a constant row added everywhere.
    out_corr = (ln_beta.astype(np.float64) @ wv2.astype(np.float64)
                @ wo.astype(np.float64)).astype(np.float32)
    return in_maps, out_corr


def kernel(visual_feat, text_feat, token_mask, Wq, Wk, Wv, Wo, ln_gamma, ln_beta):
    global LAST_RESULTS
    visual_feat = np.asarray(visual_feat, np.float32)
    text_feat = np.asarray(text_feat, np.float32)
    token_mask = np.asarray(token_mask)

    in_maps, out_corr = prep_core_inputs(
        visual_feat, text_feat, token_mask,
        np.asarray(Wq, np.float32), np.asarray(Wk, np.float32),
        np.asarray(Wv, np.float32), np.asarray(Wo, np.float32),
        np.asarray(ln_gamma, np.float32), np.asarray(ln_beta, np.float32),
    )
    nc = _get_program()
    res = run_bass_kernel_spmd(nc, in_maps, core_ids=list(range(N_CORES)))
    LAST_RESULTS = res
    out = np.stack([res.results[b]["out"] for b in range(B)], axis=0)
    if np.any(out_corr):
        out = out + out_corr[None, None, :]
    return out.reshape(B, T, S_, D)


# revision 15
# speedup vs baseline: 4.4830x; 1.2722x over previous
"""Fused cross-attention kernel for Trainium2, data-parallel over batch on 8 cores.

Per core (one batch element):
  tn   = LayerNorm(text)                      (gamma folded into Wk/Wv on host)
  Q^T  = Wq^T @ X^T   (X^T pre-transposed on host, bf16)
  K^T  = Wk^T @ tn^T,  V = tn @ Wv            (tn^T via PE transpose)
  per q-tile of 128 rows, per head:
    S    = Q_h^T.T @ K_h^T                    (q on partitions, l on free)
    E    = exp(S * scale)                     (no max-sub: |S*scale| < 2)
    A    = (E * mask) / sum(E * mask)         (masked softmax)
    C^T  = V_h.T'd via lhsT=V_h, rhs=A^T      (A^T via PE transpose)
  out  = concat_h(C) @ Wo                     (natural layout, DMA out)
"""

import sys

sys.path.insert(0, "/opt/trn_rl_repo")

import numpy as np
import ml_dtypes

import concourse.bass as bass
import concourse.mybir as mybir
import concourse.tile as tile
from concourse import bacc
from concourse.bass_utils import run_bass_kernel_spmd
from concourse.masks import make_identity

N_CORES = 8
B, T, S_, D, L, H = 8, 64, 196, 512, 77, 4
DH = D // H  # 128
NQ = T * S_  # 12544
LN_EPS = 1e-6
SCALE = float(DH) ** -0.5
P = 128
NCH = D // P  # 4 chunks of the feature dim

F32 = mybir.dt.float32
BF16 = mybir.dt.bfloat16

LAST_RESULTS = None  # BassKernelResults of the most recent run (for test harness)
_PROGRAM_CACHE = {}


def build_program(nq=NQ):
    """One SPMD program; all 8 cores run it on their own batch element.

    Uses the low-rank structure of cross-attention (L=77 << D=512):
      W2_h = Wq_h @ K_h^T   [512, 77]  -> scores_h = X @ W2_h
      W3_h = V_h @ Wo_h     [77, 512]  -> out = sum_h attn_h^T.T @ W3_h
    so the per-token work is one [512 x 308] and one [308 x 512] matmul plus
    softmax, with no Q/ctx PSUM round-trips.
    """
    nc = bacc.Bacc("TRN2", target_bir_lowering=False, debug=False, num_devices=N_CORES)

    xt = nc.dram_tensor("xt", [D, nq], BF16, kind="ExternalInput").ap()
    text = nc.dram_tensor("text", [P, D], F32, kind="ExternalInput").ap()
    maskb = nc.dram_tensor("maskb", [P], BF16, kind="ExternalInput").ap()
    negcnt = nc.dram_tensor("negcnt", [P, 1], F32, kind="ExternalInput").ap()
    wq = nc.dram_tensor("wq", [D, D], BF16, kind="ExternalInput").ap()
    wk = nc.dram_tensor("wk", [D, D], BF16, kind="ExternalInput").ap()
    wv = nc.dram_tensor("wv", [D, D], BF16, kind="ExternalInput").ap()
    wo = nc.dram_tensor("wo", [D, D], BF16, kind="ExternalInput").ap()
    out = nc.dram_tensor("out", [nq, D], F32, kind="ExternalOutput").ap()

    ntiles = nq // P
    groups = []
    t0 = 0
    while t0 < ntiles:
        gt = min(4, ntiles - t0)
        groups.append((t0, gt))
        t0 += gt

    with tile.TileContext(nc) as tc:
        with (
            tc.tile_pool(name="const", bufs=1) as const,
            tc.tile_pool(name="xtp", bufs=3) as xtp,
            tc.tile_pool(name="attp", bufs=3) as attp,
            tc.tile_pool(name="smalls", bufs=24) as smalls,
            tc.tile_pool(name="outp", bufs=3) as outp,
            tc.tile_pool(name="ps_sc", bufs=3, space="PSUM") as ps_sc,
            tc.tile_pool(name="ps_at", bufs=2, space="PSUM") as ps_at,
            tc.tile_pool(name="ps_out", bufs=3, space="PSUM") as ps_out,
        ):
            # ---- constants / prolog ----
            ident = const.tile([P, P], BF16)
            make_identity(nc, ident)

            wq_sb = const.tile([P, NCH, D], BF16, tag="wq")
            wk_sb = const.tile([P, NCH, D], BF16, tag="wk")
            wv_sb = const.tile([P, NCH, D], BF16, tag="wv")
            wo_sb = const.tile([P, NCH, D], BF16, tag="wo")
            for w_sb, w_dram in ((wq_sb, wq), (wk_sb, wk), (wv_sb, wv), (wo_sb, wo)):
                nc.sync.dma_start(
                    out=w_sb[:], in_=w_dram.rearrange("(c p) n -> p c n", p=P)
                )

            text_sb = const.tile([P, D], F32, tag="text")
            nc.sync.dma_start(out=text_sb[:], in_=text)
            negcnt_sb = const.tile([P, 1], F32, tag="negcnt")
            nc.sync.dma_start(out=negcnt_sb[:], in_=negcnt)
            maskb_sb = const.tile([P, P], BF16, tag="maskb")
            maskb_bcast = bass.AP(
                tensor=maskb.tensor, offset=maskb.offset, ap=[[0, P]] + maskb.ap
            )
            nc.gpsimd.dma_start(out=maskb_sb[:], in_=maskb_bcast)

            # ---- LayerNorm of text (77 valid rows; pad rows are zeros) ----
            stats = smalls.tile([P, 6], F32, tag="bnstats")
            nc.vector.bn_stats(out=stats[:], in_=text_sb[:])
            mv = smalls.tile([P, 2], F32, tag="bnaggr")
            nc.vector.bn_aggr(out=mv[:], in_=stats[:])
            eps_sb = smalls.tile([P, 1], F32, tag="eps")
            nc.vector.memset(eps_sb[:], LN_EPS)
            std = smalls.tile([P, 1], F32, tag="std")
            nc.scalar.activation(
                std[:], mv[:, 1:2], mybir.ActivationFunctionType.Sqrt, bias=eps_sb[:]
            )
            rstd = smalls.tile([P, 1], F32, tag="rstd")
            nc.vector.reciprocal(rstd[:], std[:])
            tn_sb = const.tile([P, D], BF16, tag="tn")
            nc.vector.tensor_scalar(
                out=tn_sb[:],
                in0=text_sb[:],
                scalar1=mv[:, 0:1],
                scalar2=rstd[:],
                op0=mybir.AluOpType.subtract,
                op1=mybir.AluOpType.mult,
            )

            # ---- tn^T (D on partitions) ----
            tnT_sb = const.tile([P, NCH, P], BF16, tag="tnT")
            for c in range(NCH):
                ps_t = ps_at.tile([P, H * P], BF16, tag="ps_trb2")
                nc.tensor.transpose(ps_t[:, :P], tn_sb[:, c * P : (c + 1) * P], ident[:])
                nc.scalar.copy(tnT_sb[:, c, :], ps_t[:, :P])

            # ---- K^T (masked cols) and V^T, feature dim on partitions ----
            kt_sb = const.tile([P, H, L], BF16, tag="kt")
            vt_sb = const.tile([P, H, L], BF16, tag="vt")
            for w_sb_, dst in ((wk_sb, kt_sb), (wv_sb, vt_sb)):
                for dch in range(NCH):
                    ps_k = ps_sc.tile([P, L], F32, tag="ps_s")
                    for kc in range(NCH):
                        nc.tensor.matmul(
                            ps_k[:],
                            w_sb_[:, kc, dch * P : (dch + 1) * P],
                            tnT_sb[:, kc, :L],
                            start=(kc == 0),
                            stop=(kc == NCH - 1),
                        )
                    nc.vector.tensor_mul(dst[:, dch, :], ps_k[:], maskb_sb[:, :L])

            # ---- Wq_h^T via PE transpose ----
            wqT_sb = const.tile([P, H, NCH, P], BF16, tag="wqT")
            for h in range(H):
                for kc in range(NCH):
                    ps_t = ps_at.tile([P, H * P], BF16, tag="ps_trb2")
                    nc.tensor.transpose(
                        ps_t[:, :P], wq_sb[:, kc, h * P : (h + 1) * P], ident[:]
                    )
                    nc.scalar.copy(wqT_sb[:, h, kc, :], ps_t[:, :P])

            # ---- W2_h = Wq_h @ K_h^T   [D, L] per head ----
            w2_sb = const.tile([P, NCH, H, L], BF16, tag="w2")
            for h in range(H):
                for dch in range(NCH):
                    ps_w = ps_sc.tile([P, L], F32, tag="ps_s")
                    nc.tensor.matmul(
                        ps_w[:], wqT_sb[:, h, dch, :], kt_sb[:, h, :],
                        start=True, stop=True,
                    )
                    nc.vector.tensor_copy(w2_sb[:, dch, h, :], ps_w[:])

            # ---- W3_h = V_h @ Wo_h   [L, D] per head ----
            w3_sb = const.tile([P, H, D], BF16, tag="w3")
            for h in range(H):
                ps_w = ps_out.tile([P, D], F32, tag="ps_o")
                nc.tensor.matmul(
                    ps_w[:L], vt_sb[:, h, :], wo_sb[:, h, :], start=True, stop=True
                )
                nc.scalar.copy(w3_sb[:L, h, :], ps_w[:L])

            # ---- main loop ----
            for t0, gt in groups:
                qg = gt * P
                q0 = t0 * P

                xt_sb = xtp.tile([P, NCH, qg], BF16, tag="xt")
                nc.sync.dma_start(
                    out=xt_sb[:],
                    in_=xt.rearrange("(c p) q -> p c q", p=P)[:, :, q0 : q0 + qg],
                )

                for t in range(gt):
                    tq = slice(t * P, (t + 1) * P)
                    ps_s = ps_sc.tile([P, H * L], F32, tag="ps_s")
                    for h in range(H):
                        for kc in range(NCH):
                            nc.tensor.matmul(
                                ps_s[:, h * L : (h + 1) * L],
                                xt_sb[:, kc, tq],
                                w2_sb[:, kc, h, :],
                                start=(kc == 0),
                                stop=(kc == NCH - 1),
                            )
                    exp_sb = attp.tile([P, H, L], BF16, tag="exp")
                    nc.scalar.activation(
                        exp_sb[:], ps_s[:].rearrange("p (c n) -> p c n", c=H),
                        mybir.ActivationFunctionType.Exp, scale=SCALE,
                    )
                    sumexp = smalls.tile([P, H], F32, tag="sumexp")
                    nc.vector.reduce_sum(
                        out=sumexp[:], in_=exp_sb[:], axis=mybir.AxisListType.X
                    )
                    sumadj = smalls.tile([P, H], F32, tag="sumadj")
                    nc.vector.tensor_scalar_add(sumadj[:], sumexp[:], negcnt_sb[:])
                    recip = smalls.tile([P, H], F32, tag="recip")
                    nc.vector.reciprocal_approx_fast(recip[:], sumadj[:])
                    attn_sb = attp.tile([P, H, L], BF16, tag="attn")
                    nc.vector.tensor_mul(
                        attn_sb[:], exp_sb[:], recip[:].to_broadcast([P, H, L])
                    )
                    ps_a = ps_at.tile([P, H * P], BF16, tag="ps_trb2")
                    for h in range(H):
                        nc.tensor.transpose(
                            ps_a[:L, h * P : (h + 1) * P], attn_sb[:, h, :], ident[:]
                        )
                    attnT_sb = attp.tile([P, H, P], BF16, tag="attnT")
                    nc.scalar.copy(
                        attnT_sb[:L], ps_a[:L].rearrange("p (c n) -> p c n", c=H)
                    )
                    ps_o = ps_out.tile([P, D], F32, tag="ps_o")
                    for h in range(H):
                        nc.tensor.matmul(
                            ps_o[:],
                            attnT_sb[:L, h, :],
                            w3_sb[:L, h, :],
                            start=(h == 0),
                            stop=(h == H - 1),
                        )
                    out_sb = outp.tile([P, D], F32, tag="out")
                    nc.vector.tensor_copy(out_sb[:], ps_o[:])
                    nc.sync.dma_start(
                        out=out[q0 + t * P : q0 + (t + 1) * P, :], in_=out_sb[:]
                    )

    nc.compile()
    return nc


def _get_program(nq=NQ):
    if nq not in _PROGRAM_CACHE:
        _PROGRAM_CACHE[nq] = build_program(nq)
    return _PROGRAM_CACHE[nq]


def prep_core_inputs(visual_feat, text_feat, token_mask, wq, wk, wv, wo,
                     ln_gamma, ln_beta):
    """Host-side prep: shard over batch, fold gamma, transpose X, cast bf16."""
    vf = np.ascontiguousarray(visual_feat.reshape(B, -1, D))
    wk2 = (ln_gamma[:, None] * wk).astype(np.float32)
    wv2 = (ln_gamma[:, None] * wv).astype(np.float32)
    wq_b = wq.astype(ml_dtypes.bfloat16)
    wk_b = wk2.astype(ml_dtypes.bfloat16)
    wv_b = wv2.astype(ml_dtypes.bfloat16)
    wo_b = wo.astype(ml_dtypes.bfloat16)

    in_maps = []
    for b in range(B):
        xt = np.ascontiguousarray(vf[b].T).astype(ml_dtypes.bfloat16)
        text = np.zeros((P, D), np.float32)
        text[:L] = text_feat[b]
        m = token_mask[b].astype(np.float32)
        maskb = np.zeros((P,), ml_dtypes.bfloat16)
        maskb[:L] = m.astype(ml_dtypes.bfloat16)
        negcnt = np.full((P, 1), -(L - float(m.sum())), np.float32)
        in_maps.append({
            "xt": xt, "text": text, "maskb": maskb, "negcnt": negcnt,
            "wq": wq_b, "wk": wk_b, "wv": wv_b, "wo": wo_b,
        })
    # LN beta correction: beta affects scores only via a softmax-invariant
    # per-row constant, and the output via a constant row added everywhere.
    out_corr = (ln_beta.astype(np.float64) @ wv2.astype(np.float64)
                @ wo.astype(np.float64)).astype(np.float32)
    return in_maps, out_corr


def kernel(visual_feat, text_feat, token_mask, Wq, Wk, Wv, Wo, ln_gamma, ln_beta):
    global LAST_RESULTS
    visual_feat = np.asarray(visual_feat, np.float32)
    text_feat = np.asarray(text_feat, np.float32)
    token_mask = np.asarray(token_mask)

    in_maps, out_corr = prep_core_inputs(
        visual_feat, text_feat, token_mask,
        np.asarray(Wq, np.float32), np.asarray(Wk, np.float32),
        np.asarray(Wv, np.float32), np.asarray(Wo, np.float32),
        np.asarray(ln_gamma, np.float32), np.asarray(ln_beta, np.float32),
    )
    nc = _get_program()
    res = run_bass_kernel_spmd(nc, in_maps, core_ids=list(range(N_CORES)))
    LAST_RESULTS = res
    out = np.stack([res.results[b]["out"] for b in range(B)], axis=0)
    if np.any(out_corr):
        out = out + out_corr[None, None, :]
    return out.reshape(B, T, S_, D)


# revision 16
# speedup vs baseline: 5.0722x; 1.1314x over previous
"""Fused cross-attention kernel for Trainium2, data-parallel over batch on 8 cores.

Per core (one batch element):
  tn   = LayerNorm(text)                      (gamma folded into Wk/Wv on host)
  Q^T  = Wq^T @ X^T   (X^T pre-transposed on host, bf16)
  K^T  = Wk^T @ tn^T,  V = tn @ Wv            (tn^T via PE transpose)
  per q-tile of 128 rows, per head:
    S    = Q_h^T.T @ K_h^T                    (q on partitions, l on free)
    E    = exp(S * scale)                     (no max-sub: |S*scale| < 2)
    A    = (E * mask) / sum(E * mask)         (masked softmax)
    C^T  = V_h.T'd via lhsT=V_h, rhs=A^T      (A^T via PE transpose)
  out  = concat_h(C) @ Wo                     (natural layout, DMA out)
"""

import sys

sys.path.insert(0, "/opt/trn_rl_repo")

import numpy as np
import ml_dtypes

import concourse.bass as bass
import concourse.mybir as mybir
import concourse.tile as tile
from concourse import bacc
from concourse.bass_utils import run_bass_kernel_spmd
from concourse.masks import make_identity

N_CORES = 8
B, T, S_, D, L, H = 8, 64, 196, 512, 77, 4
DH = D // H  # 128
NQ = T * S_  # 12544
LN_EPS = 1e-6
SCALE = float(DH) ** -0.5
P = 128
NCH = D // P  # 4 chunks of the feature dim

F32 = mybir.dt.float32
BF16 = mybir.dt.bfloat16

LAST_RESULTS = None  # BassKernelResults of the most recent run (for test harness)
_PROGRAM_CACHE = {}


def build_program(nq=NQ):
    """One SPMD program; all 8 cores run it on their own batch element.

    Uses the low-rank structure of cross-attention (L=77 << D=512):
      W2_h = Wq_h @ K_h^T   [512, 77]  -> scores_h = X @ W2_h
      W3_h = V_h @ Wo_h     [77, 512]  -> out = sum_h attn_h^T.T @ W3_h
    so the per-token work is one [512 x 308] and one [308 x 512] matmul plus
    softmax, with no Q/ctx PSUM round-trips.
    """
    nc = bacc.Bacc("TRN2", target_bir_lowering=False, debug=False, num_devices=N_CORES)

    xt = nc.dram_tensor("xt", [D, nq], BF16, kind="ExternalInput").ap()
    text = nc.dram_tensor("text", [P, D], F32, kind="ExternalInput").ap()
    maskb = nc.dram_tensor("maskb", [P], BF16, kind="ExternalInput").ap()
    negcnt = nc.dram_tensor("negcnt", [P, 1], F32, kind="ExternalInput").ap()
    wq = nc.dram_tensor("wq", [D, D], BF16, kind="ExternalInput").ap()
    wk = nc.dram_tensor("wk", [D, D], BF16, kind="ExternalInput").ap()
    wv = nc.dram_tensor("wv", [D, D], BF16, kind="ExternalInput").ap()
    wo = nc.dram_tensor("wo", [D, D], BF16, kind="ExternalInput").ap()
    out = nc.dram_tensor("out", [nq, D], F32, kind="ExternalOutput").ap()

    ntiles = nq // P
    groups = []
    t0 = 0
    while t0 < ntiles:
        gt = min(4, ntiles - t0)
        groups.append((t0, gt))
        t0 += gt

    with tile.TileContext(nc) as tc:
        with (
            tc.tile_pool(name="const", bufs=1) as const,
            tc.tile_pool(name="xtp", bufs=3) as xtp,
            tc.tile_pool(name="attp", bufs=4) as attp,
            tc.tile_pool(name="smalls", bufs=24) as smalls,
            tc.tile_pool(name="outp", bufs=4) as outp,
            tc.tile_pool(name="ps_sc", bufs=3, space="PSUM") as ps_sc,
            tc.tile_pool(name="ps_at", bufs=2, space="PSUM") as ps_at,
            tc.tile_pool(name="ps_out", bufs=3, space="PSUM") as ps_out,
        ):
            # ---- constants / prolog ----
            ident = const.tile([P, P], BF16)
            make_identity(nc, ident)

            wq_sb = const.tile([P, NCH, D], BF16, tag="wq")
            wk_sb = const.tile([P, NCH, D], BF16, tag="wk")
            wv_sb = const.tile([P, NCH, D], BF16, tag="wv")
            wo_sb = const.tile([P, NCH, D], BF16, tag="wo")
            for w_sb, w_dram in ((wq_sb, wq), (wk_sb, wk), (wv_sb, wv), (wo_sb, wo)):
                nc.sync.dma_start(
                    out=w_sb[:], in_=w_dram.rearrange("(c p) n -> p c n", p=P)
                )

            text_sb = const.tile([P, D], F32, tag="text")
            nc.sync.dma_start(out=text_sb[:], in_=text)
            negcnt_sb = const.tile([P, 1], F32, tag="negcnt")
            nc.sync.dma_start(out=negcnt_sb[:], in_=negcnt)
            maskb_sb = const.tile([P, P], BF16, tag="maskb")
            maskb_bcast = bass.AP(
                tensor=maskb.tensor, offset=maskb.offset, ap=[[0, P]] + maskb.ap
            )
            nc.gpsimd.dma_start(out=maskb_sb[:], in_=maskb_bcast)

            # ---- LayerNorm of text (77 valid rows; pad rows are zeros) ----
            stats = smalls.tile([P, 6], F32, tag="bnstats")
            nc.vector.bn_stats(out=stats[:], in_=text_sb[:])
            mv = smalls.tile([P, 2], F32, tag="bnaggr")
            nc.vector.bn_aggr(out=mv[:], in_=stats[:])
            eps_sb = smalls.tile([P, 1], F32, tag="eps")
            nc.vector.memset(eps_sb[:], LN_EPS)
            std = smalls.tile([P, 1], F32, tag="std")
            nc.scalar.activation(
                std[:], mv[:, 1:2], mybir.ActivationFunctionType.Sqrt, bias=eps_sb[:]
            )
            rstd = smalls.tile([P, 1], F32, tag="rstd")
            nc.vector.reciprocal(rstd[:], std[:])
            tn_sb = const.tile([P, D], BF16, tag="tn")
            nc.vector.tensor_scalar(
                out=tn_sb[:],
                in0=text_sb[:],
                scalar1=mv[:, 0:1],
                scalar2=rstd[:],
                op0=mybir.AluOpType.subtract,
                op1=mybir.AluOpType.mult,
            )

            # ---- tn^T (D on partitions) ----
            tnT_sb = const.tile([P, NCH, P], BF16, tag="tnT")
            for c in range(NCH):
                ps_t = ps_at.tile([P, H * P], BF16, tag="ps_trb2")
                nc.tensor.transpose(ps_t[:, :P], tn_sb[:, c * P : (c + 1) * P], ident[:])
                nc.scalar.copy(tnT_sb[:, c, :], ps_t[:, :P])

            # ---- K^T (masked cols) and V^T, feature dim on partitions ----
            kt_sb = const.tile([P, H, L], BF16, tag="kt")
            vt_sb = const.tile([P, H, L], BF16, tag="vt")
            for w_sb_, dst in ((wk_sb, kt_sb), (wv_sb, vt_sb)):
                for dch in range(NCH):
                    ps_k = ps_sc.tile([P, L], F32, tag="ps_s")
                    for kc in range(NCH):
                        nc.tensor.matmul(
                            ps_k[:],
                            w_sb_[:, kc, dch * P : (dch + 1) * P],
                            tnT_sb[:, kc, :L],
                            start=(kc == 0),
                            stop=(kc == NCH - 1),
                        )
                    nc.vector.tensor_mul(dst[:, dch, :], ps_k[:], maskb_sb[:, :L])

            # ---- Wq_h^T via PE transpose ----
            wqT_sb = const.tile([P, H, NCH, P], BF16, tag="wqT")
            for h in range(H):
                for kc in range(NCH):
                    ps_t = ps_at.tile([P, H * P], BF16, tag="ps_trb2")
                    nc.tensor.transpose(
                        ps_t[:, :P], wq_sb[:, kc, h * P : (h + 1) * P], ident[:]
                    )
                    nc.scalar.copy(wqT_sb[:, h, kc, :], ps_t[:, :P])

            # ---- W2_h = Wq_h @ K_h^T   [D, L] per head ----
            w2_sb = const.tile([P, NCH, H, L], BF16, tag="w2")
            for h in range(H):
                for dch in range(NCH):
                    ps_w = ps_sc.tile([P, L], F32, tag="ps_s")
                    nc.tensor.matmul(
                        ps_w[:], wqT_sb[:, h, dch, :], kt_sb[:, h, :],
                        start=True, stop=True,
                    )
                    nc.vector.tensor_copy(w2_sb[:, dch, h, :], ps_w[:])

            # ---- W3_h = V_h @ Wo_h   [L, D] per head ----
            w3_sb = const.tile([P, H, D], BF16, tag="w3")
            for h in range(H):
                ps_w = ps_out.tile([P, D], F32, tag="ps_o")
                nc.tensor.matmul(
                    ps_w[:L], vt_sb[:, h, :], wo_sb[:, h, :], start=True, stop=True
                )
                nc.scalar.copy(w3_sb[:L, h, :], ps_w[:L])

            # ---- main loop ----
            for t0, gt in groups:
                qg = gt * P
                q0 = t0 * P

                xt_sb = xtp.tile([P, NCH, qg], BF16, tag="xt")
                nc.sync.dma_start(
                    out=xt_sb[:],
                    in_=xt.rearrange("(c p) q -> p c q", p=P)[:, :, q0 : q0 + qg],
                )

                for t in range(gt):
                    tq = slice(t * P, (t + 1) * P)
                    ps_s = ps_sc.tile([P, H * L], F32, tag="ps_s")
                    for h in range(H):
                        for kc in range(NCH):
                            nc.tensor.matmul(
                                ps_s[:, h * L : (h + 1) * L],
                                xt_sb[:, kc, tq],
                                w2_sb[:, kc, h, :],
                                start=(kc == 0),
                                stop=(kc == NCH - 1),
                            )
                    exp_sb = attp.tile([P, H, L], BF16, tag="exp")
                    nc.scalar.activation(
                        exp_sb[:], ps_s[:].rearrange("p (c n) -> p c n", c=H),
                        mybir.ActivationFunctionType.Exp, scale=SCALE,
                    )
                    sumexp = smalls.tile([P, H], F32, tag="sumexp")
                    nc.vector.reduce_sum(
                        out=sumexp[:], in_=exp_sb[:], axis=mybir.AxisListType.X
                    )
                    sumadj = smalls.tile([P, H], F32, tag="sumadj")
                    nc.vector.tensor_scalar_add(sumadj[:], sumexp[:], negcnt_sb[:])
                    recip = smalls.tile([P, H], F32, tag="recip")
                    nc.vector.reciprocal_approx_fast(recip[:], sumadj[:])
                    recip_b = smalls.tile([P, H], BF16, tag="recip_b")
                    nc.vector.tensor_copy(recip_b[:], recip[:])
                    attn_sb = attp.tile([P, H, L], BF16, tag="attn")
                    nc.vector.tensor_mul(
                        attn_sb[:], exp_sb[:], recip_b[:].to_broadcast([P, H, L])
                    )
                    ps_a = ps_at.tile([P, H * P], BF16, tag="ps_trb2")
                    for h in range(H):
                        nc.tensor.transpose(
                            ps_a[:L, h * P : (h + 1) * P], attn_sb[:, h, :], ident[:]
                        )
                    attnT_sb = attp.tile([P, H, P], BF16, tag="attnT")
                    at_eng = nc.scalar.copy if (t0 + t) % 2 == 0 else nc.vector.tensor_copy
                    at_eng(attnT_sb[:L], ps_a[:L].rearrange("p (c n) -> p c n", c=H))
                    ps_o = ps_out.tile([P, D], F32, tag="ps_o")
                    for h in range(H):
                        nc.tensor.matmul(
                            ps_o[:],
                            attnT_sb[:L, h, :],
                            w3_sb[:L, h, :],
                            start=(h == 0),
                            stop=(h == H - 1),
                        )
                    out_sb = outp.tile([P, D], F32, tag="out")
                    o_eng = nc.vector.tensor_copy if (t0 + t) % 2 == 0 else nc.scalar.copy
                    o_eng(out_sb[:], ps_o[:])
                    nc.sync.dma_start(
                        out=out[q0 + t * P : q0 + (t + 1) * P, :], in_=out_sb[:]
                    )

    nc.compile()
    return nc


def _get_program(nq=NQ):
    if nq not in _PROGRAM_CACHE:
        _PROGRAM_CACHE[nq] = build_program(nq)
    return _PROGRAM_CACHE[nq]


def prep_core_inputs(visual_feat, text_feat, token_mask, wq, wk, wv, wo,
                     ln_gamma, ln_beta):
    """Host-side prep: shard over batch, fold gamma, transpose X, cast bf16."""
    vf = np.ascontiguousarray(visual_feat.reshape(B, -1, D))
    wk2 = (ln_gamma[:, None] * wk).astype(np.float32)
    wv2 = (ln_gamma[:, None] * wv).astype(np.float32)
    wq_b = wq.astype(ml_dtypes.bfloat16)
    wk_b = wk2.astype(ml_dtypes.bfloat16)
    wv_b = wv2.astype(ml_dtypes.bfloat16)
    wo_b = wo.astype(ml_dtypes.bfloat16)

    in_maps = []
    for b in range(B):
        xt = np.ascontiguousarray(vf[b].T).astype(ml_dtypes.bfloat16)
        text = np.zeros((P, D), np.float32)
        text[:L] = text_feat[b]
        m = token_mask[b].astype(np.float32)
        maskb = np.zeros((P,), ml_dtypes.bfloat16)
        maskb[:L] = m.astype(ml_dtypes.bfloat16)
        negcnt = np.full((P, 1), -(L - float(m.sum())), np.float32)
        in_maps.append({
            "xt": xt, "text": text, "maskb": maskb, "negcnt": negcnt,
            "wq": wq_b, "wk": wk_b, "wv": wv_b, "wo": wo_b,
        })
    # LN beta correction: beta affects scores only via a softmax-invariant
    # per-row constant, and the output via a constant row added everywhere.
    out_corr = (ln_beta.astype(np.float64) @ wv2.astype(np.float64)
                @ wo.astype(np.float64)).astype(np.float32)
    return in_maps, out_corr


def kernel(visual_feat, text_feat, token_mask, Wq, Wk, Wv, Wo, ln_gamma, ln_beta):
    global LAST_RESULTS
    visual_feat = np.asarray(visual_feat, np.float32)
    text_feat = np.asarray(text_feat, np.float32)
    token_mask = np.asarray(token_mask)

    in_maps, out_corr = prep_core_inputs(
        visual_feat, text_feat, token_mask,
        np.asarray(Wq, np.float32), np.asarray(Wk, np.float32),
        np.asarray(Wv, np.float32), np.asarray(Wo, np.float32),
        np.asarray(ln_gamma, np.float32), np.asarray(ln_beta, np.float32),
    )
    nc = _get_program()
    res = run_bass_kernel_spmd(nc, in_maps, core_ids=list(range(N_CORES)))
    LAST_RESULTS = res
    out = np.stack([res.results[b]["out"] for b in range(B)], axis=0)
    if np.any(out_corr):
        out = out + out_corr[None, None, :]
    return out.reshape(B, T, S_, D)


# revision 18
# speedup vs baseline: 5.0859x; 1.0027x over previous
"""Fused cross-attention kernel for Trainium2, data-parallel over batch on 8 cores.

Per core (one batch element):
  tn   = LayerNorm(text)                      (gamma folded into Wk/Wv on host)
  Q^T  = Wq^T @ X^T   (X^T pre-transposed on host, bf16)
  K^T  = Wk^T @ tn^T,  V = tn @ Wv            (tn^T via PE transpose)
  per q-tile of 128 rows, per head:
    S    = Q_h^T.T @ K_h^T                    (q on partitions, l on free)
    E    = exp(S * scale)                     (no max-sub: |S*scale| < 2)
    A    = (E * mask) / sum(E * mask)         (masked softmax)
    C^T  = V_h.T'd via lhsT=V_h, rhs=A^T      (A^T via PE transpose)
  out  = concat_h(C) @ Wo                     (natural layout, DMA out)
"""

import sys

sys.path.insert(0, "/opt/trn_rl_repo")

import numpy as np
import ml_dtypes

import concourse.bass as bass
import concourse.mybir as mybir
import concourse.tile as tile
from concourse import bacc
from concourse.bass_utils import run_bass_kernel_spmd
from concourse.masks import make_identity

N_CORES = 8
B, T, S_, D, L, H = 8, 64, 196, 512, 77, 4
DH = D // H  # 128
NQ = T * S_  # 12544
LN_EPS = 1e-6
SCALE = float(DH) ** -0.5
P = 128
NCH = D // P  # 4 chunks of the feature dim

F32 = mybir.dt.float32
BF16 = mybir.dt.bfloat16

LAST_RESULTS = None  # BassKernelResults of the most recent run (for test harness)
_PROGRAM_CACHE = {}


def build_program(nq=NQ):
    """One SPMD program; all 8 cores run it on their own batch element.

    Uses the low-rank structure of cross-attention (L=77 << D=512):
      W2_h = Wq_h @ K_h^T   [512, 77]  -> scores_h = X @ W2_h
      W3_h = V_h @ Wo_h     [77, 512]  -> out = sum_h attn_h^T.T @ W3_h
    so the per-token work is one [512 x 308] and one [308 x 512] matmul plus
    softmax, with no Q/ctx PSUM round-trips.
    """
    nc = bacc.Bacc("TRN2", target_bir_lowering=False, debug=False, num_devices=N_CORES)

    xt = nc.dram_tensor("xt", [D, nq], BF16, kind="ExternalInput").ap()
    text = nc.dram_tensor("text", [P, D], F32, kind="ExternalInput").ap()
    maskb = nc.dram_tensor("maskb", [P], BF16, kind="ExternalInput").ap()
    negcnt = nc.dram_tensor("negcnt", [P, 1], F32, kind="ExternalInput").ap()
    wq = nc.dram_tensor("wq", [D, D], BF16, kind="ExternalInput").ap()
    wk = nc.dram_tensor("wk", [D, D], BF16, kind="ExternalInput").ap()
    wv = nc.dram_tensor("wv", [D, D], BF16, kind="ExternalInput").ap()
    wo = nc.dram_tensor("wo", [D, D], BF16, kind="ExternalInput").ap()
    out = nc.dram_tensor("out", [nq, D], BF16, kind="ExternalOutput").ap()

    ntiles = nq // P
    groups = []
    t0 = 0
    while t0 < ntiles:
        gt = min(4, ntiles - t0)
        groups.append((t0, gt))
        t0 += gt

    with tile.TileContext(nc) as tc:
        with (
            tc.tile_pool(name="const", bufs=1) as const,
            tc.tile_pool(name="xtp", bufs=3) as xtp,
            tc.tile_pool(name="attp", bufs=4) as attp,
            tc.tile_pool(name="smalls", bufs=24) as smalls,
            tc.tile_pool(name="outp", bufs=4) as outp,
            tc.tile_pool(name="ps_sc", bufs=3, space="PSUM") as ps_sc,
            tc.tile_pool(name="ps_at", bufs=2, space="PSUM") as ps_at,
            tc.tile_pool(name="ps_out", bufs=3, space="PSUM") as ps_out,
        ):
            # ---- constants / prolog ----
            ident = const.tile([P, P], BF16)
            make_identity(nc, ident)

            wq_sb = const.tile([P, NCH, D], BF16, tag="wq")
            wk_sb = const.tile([P, NCH, D], BF16, tag="wk")
            wv_sb = const.tile([P, NCH, D], BF16, tag="wv")
            wo_sb = const.tile([P, NCH, D], BF16, tag="wo")
            for w_sb, w_dram in ((wq_sb, wq), (wk_sb, wk), (wv_sb, wv), (wo_sb, wo)):
                nc.sync.dma_start(
                    out=w_sb[:], in_=w_dram.rearrange("(c p) n -> p c n", p=P)
                )

            text_sb = const.tile([P, D], F32, tag="text")
            nc.sync.dma_start(out=text_sb[:], in_=text)
            negcnt_sb = const.tile([P, 1], F32, tag="negcnt")
            nc.sync.dma_start(out=negcnt_sb[:], in_=negcnt)
            maskb_sb = const.tile([P, P], BF16, tag="maskb")
            maskb_bcast = bass.AP(
                tensor=maskb.tensor, offset=maskb.offset, ap=[[0, P]] + maskb.ap
            )
            nc.gpsimd.dma_start(out=maskb_sb[:], in_=maskb_bcast)

            # ---- LayerNorm of text (77 valid rows; pad rows are zeros) ----
            stats = smalls.tile([P, 6], F32, tag="bnstats")
            nc.vector.bn_stats(out=stats[:], in_=text_sb[:])
            mv = smalls.tile([P, 2], F32, tag="bnaggr")
            nc.vector.bn_aggr(out=mv[:], in_=stats[:])
            eps_sb = smalls.tile([P, 1], F32, tag="eps")
            nc.vector.memset(eps_sb[:], LN_EPS)
            std = smalls.tile([P, 1], F32, tag="std")
            nc.scalar.activation(
                std[:], mv[:, 1:2], mybir.ActivationFunctionType.Sqrt, bias=eps_sb[:]
            )
            rstd = smalls.tile([P, 1], F32, tag="rstd")
            nc.vector.reciprocal(rstd[:], std[:])
            tn_sb = const.tile([P, D], BF16, tag="tn")
            nc.vector.tensor_scalar(
                out=tn_sb[:],
                in0=text_sb[:],
                scalar1=mv[:, 0:1],
                scalar2=rstd[:],
                op0=mybir.AluOpType.subtract,
                op1=mybir.AluOpType.mult,
            )

            # ---- tn^T (D on partitions) ----
            tnT_sb = const.tile([P, NCH, P], BF16, tag="tnT")
            for c in range(NCH):
                ps_t = ps_at.tile([P, H * P], BF16, tag="ps_trb2")
                nc.tensor.transpose(ps_t[:, :P], tn_sb[:, c * P : (c + 1) * P], ident[:])
                nc.scalar.copy(tnT_sb[:, c, :], ps_t[:, :P])

            # ---- K^T (masked cols) and V^T, feature dim on partitions ----
            kt_sb = const.tile([P, H, L], BF16, tag="kt")
            vt_sb = const.tile([P, H, L], BF16, tag="vt")
            for w_sb_, dst in ((wk_sb, kt_sb), (wv_sb, vt_sb)):
                for dch in range(NCH):
                    ps_k = ps_sc.tile([P, L], F32, tag="ps_s")
                    for kc in range(NCH):
                        nc.tensor.matmul(
                            ps_k[:],
                            w_sb_[:, kc, dch * P : (dch + 1) * P],
                            tnT_sb[:, kc, :L],
                            start=(kc == 0),
                            stop=(kc == NCH - 1),
                        )
                    nc.vector.tensor_mul(dst[:, dch, :], ps_k[:], maskb_sb[:, :L])

            # ---- Wq_h^T via PE transpose ----
            wqT_sb = const.tile([P, H, NCH, P], BF16, tag="wqT")
            for h in range(H):
                for kc in range(NCH):
                    ps_t = ps_at.tile([P, H * P], BF16, tag="ps_trb2")
                    nc.tensor.transpose(
                        ps_t[:, :P], wq_sb[:, kc, h * P : (h + 1) * P], ident[:]
                    )
                    nc.scalar.copy(wqT_sb[:, h, kc, :], ps_t[:, :P])

            # ---- W2_h = Wq_h @ K_h^T   [D, L] per head ----
            w2_sb = const.tile([P, NCH, H, L], BF16, tag="w2")
            for h in range(H):
                for dch in range(NCH):
                    ps_w = ps_sc.tile([P, L], F32, tag="ps_s")
                    nc.tensor.matmul(
                        ps_w[:], wqT_sb[:, h, dch, :], kt_sb[:, h, :],
                        start=True, stop=True,
                    )
                    nc.vector.tensor_copy(w2_sb[:, dch, h, :], ps_w[:])

            # ---- W3_h = V_h @ Wo_h   [L, D] per head ----
            w3_sb = const.tile([P, H, D], BF16, tag="w3")
            for h in range(H):
                ps_w = ps_out.tile([P, D], F32, tag="ps_o")
                nc.tensor.matmul(
                    ps_w[:L], vt_sb[:, h, :], wo_sb[:, h, :], start=True, stop=True
                )
                nc.scalar.copy(w3_sb[:L, h, :], ps_w[:L])

            # ---- main loop ----
            for t0, gt in groups:
                qg = gt * P
                q0 = t0 * P

                xt_sb = xtp.tile([P, NCH, qg], BF16, tag="xt")
                nc.sync.dma_start(
                    out=xt_sb[:],
                    in_=xt.rearrange("(c p) q -> p c q", p=P)[:, :, q0 : q0 + qg],
                )

                for t in range(gt):
                    tq = slice(t * P, (t + 1) * P)
                    ps_s = ps_sc.tile([P, H * L], F32, tag="ps_s")
                    for kc in range(NCH):
                        nc.tensor.matmul(
                            ps_s[:],
                            xt_sb[:, kc, tq],
                            w2_sb[:, kc, :, :].rearrange("p c n -> p (c n)"),
                            start=(kc == 0),
                            stop=(kc == NCH - 1),
                        )
                    exp_sb = attp.tile([P, H, L], BF16, tag="exp")
                    nc.scalar.activation(
                        exp_sb[:], ps_s[:].rearrange("p (c n) -> p c n", c=H),
                        mybir.ActivationFunctionType.Exp, scale=SCALE,
                    )
                    sumexp = smalls.tile([P, H], F32, tag="sumexp")
                    nc.vector.reduce_sum(
                        out=sumexp[:], in_=exp_sb[:], axis=mybir.AxisListType.X
                    )
                    sumadj = smalls.tile([P, H], F32, tag="sumadj")
                    nc.vector.tensor_scalar_add(sumadj[:], sumexp[:], negcnt_sb[:])
                    recip = smalls.tile([P, H], F32, tag="recip")
                    nc.vector.reciprocal_approx_fast(recip[:], sumadj[:])
                    recip_b = smalls.tile([P, H], BF16, tag="recip_b")
                    nc.vector.tensor_copy(recip_b[:], recip[:])
                    attn_sb = attp.tile([P, H, L], BF16, tag="attn")
                    nc.vector.tensor_mul(
                        attn_sb[:], exp_sb[:], recip_b[:].to_broadcast([P, H, L])
                    )
                    ps_a = ps_at.tile([P, H * P], BF16, tag="ps_trb2")
                    for h in range(H):
                        nc.tensor.transpose(
                            ps_a[:L, h * P : (h + 1) * P], attn_sb[:, h, :], ident[:]
                        )
                    attnT_sb = attp.tile([P, H, P], BF16, tag="attnT")
                    at_eng = nc.scalar.copy if (t0 + t) % 2 == 0 else nc.vector.tensor_copy
                    at_eng(attnT_sb[:L], ps_a[:L].rearrange("p (c n) -> p c n", c=H))
                    ps_o = ps_out.tile([P, D], F32, tag="ps_o")
                    for h in range(H):
                        nc.tensor.matmul(
                            ps_o[:],
                            attnT_sb[:L, h, :],
                            w3_sb[:L, h, :],
                            start=(h == 0),
                            stop=(h == H - 1),
                        )
                    out_sb = outp.tile([P, D], BF16, tag="out")
                    o_eng = nc.vector.tensor_copy if (t0 + t) % 2 == 0 else nc.scalar.copy
                    o_eng(out_sb[:], ps_o[:])
                    nc.sync.dma_start(
                        out=out[q0 + t * P : q0 + (t + 1) * P, :], in_=out_sb[:]
                    )

    nc.compile()
    return nc


def _get_program(nq=NQ):
    if nq not in _PROGRAM_CACHE:
        _PROGRAM_CACHE[nq] = build_program(nq)
    return _PROGRAM_CACHE[nq]


def prep_core_inputs(visual_feat, text_feat, token_mask, wq, wk, wv, wo,
                     ln_gamma, ln_beta):
    """Host-side prep: shard over batch, fold gamma, transpose X, cast bf16."""
    vf = np.ascontiguousarray(visual_feat.reshape(B, -1, D))
    wk2 = (ln_gamma[:, None] * wk).astype(np.float32)
    wv2 = (ln_gamma[:, None] * wv).astype(np.float32)
    wq_b = wq.astype(ml_dtypes.bfloat16)
    wk_b = wk2.astype(ml_dtypes.bfloat16)
    wv_b = wv2.astype(ml_dtypes.bfloat16)
    wo_b = wo.astype(ml_dtypes.bfloat16)

    in_maps = []
    for b in range(B):
        xt = np.ascontiguousarray(vf[b].T).astype(ml_dtypes.bfloat16)
        text = np.zeros((P, D), np.float32)
        text[:L] = text_feat[b]
        m = token_mask[b].astype(np.float32)
        maskb = np.zeros((P,), ml_dtypes.bfloat16)
        maskb[:L] = m.astype(ml_dtypes.bfloat16)
        negcnt = np.full((P, 1), -(L - float(m.sum())), np.float32)
        in_maps.append({
            "xt": xt, "text": text, "maskb": maskb, "negcnt": negcnt,
            "wq": wq_b, "wk": wk_b, "wv": wv_b, "wo": wo_b,
        })
    # LN beta correction: beta affects scores only via a softmax-invariant
    # per-row constant, and the output via a constant row added everywhere.
    out_corr = (ln_beta.astype(np.float64) @ wv2.astype(np.float64)
                @ wo.astype(np.float64)).astype(np.float32)
    return in_maps, out_corr


def kernel(visual_feat, text_feat, token_mask, Wq, Wk, Wv, Wo, ln_gamma, ln_beta):
    global LAST_RESULTS
    visual_feat = np.asarray(visual_feat, np.float32)
    text_feat = np.asarray(text_feat, np.float32)
    token_mask = np.asarray(token_mask)

    in_maps, out_corr = prep_core_inputs(
        visual_feat, text_feat, token_mask,
        np.asarray(Wq, np.float32), np.asarray(Wk, np.float32),
        np.asarray(Wv, np.float32), np.asarray(Wo, np.float32),
        np.asarray(ln_gamma, np.float32), np.asarray(ln_beta, np.float32),
    )
    nc = _get_program()
    res = run_bass_kernel_spmd(nc, in_maps, core_ids=list(range(N_CORES)))
    LAST_RESULTS = res
    out = np.stack([res.results[b]["out"].astype(np.float32) for b in range(B)], axis=0)
    if np.any(out_corr):
        out = out + out_corr[None, None, :]
    return out.reshape(B, T, S_, D)
